# revision 36
# baseline (speedup 1.0000x reference)
"""ContrastiveHardestNegativeLoss on 8 Trainium2 NeuronCores (Bass/Tile).

Strategy (per sharding hint): shard the positive-pair (row) dimension of the
P x M distance matrices across the 8 cores. Rows are DEDUPLICATED first
(matches[:,0] / matches[:,1] draw ~15.1k unique of 100k points for 16.4k
pairs), padded with zero-rows to 8*1920; each core gets 15 row-tiles of 128
per matrix instead of 16 (-6.25% work). Each core receives:
  - its slice of the deduped pos features, transposed + augmented:
      lhs[d, i] = posF[i, d] for d < 32, lhs[32, i] = 1.0, lhs[33, i] = |p_i|^2
  - the full gathered sub features, transposed, scaled and augmented:
      rhs[d, c] = -2 * subF[c, d],  rhs[32, c] = |subF[c]|^2,  rhs[33, c] = 1.0
  so a single K=34 bf16 PE matmul produces the FULL squared distance
  q[i, c] = |p_i|^2 + |s_c|^2 - 2 <p_i, s_c>  (K<=64 so the extra rows are
  free: matmul cost depends only on output columns).

The hardest-negative terms are exactly zero whenever every distance exceeds
NEG_THRESH (true with huge margin here: min distance ~2.9 vs 1.4), in which
case the pair-mask cannot affect the result. The kernel therefore only needs
a *certificate* that min d^2 > TH = 3.0 > NEG_THRESH^2 + bf16 error margin;
if the certificate fails (or the input's unique-row counts exceed the static
padding) it falls back to an exact host recompute.

PSUM egress is the roofline (only ScalarE and VectorE have PSUM read ports):
granules of 1024 cols are consumed in pairs - ScalarE copies the EVEN granule
to SBUF, then the 2-stream custom DVE min op consumes (odd PSUM, even copy)
at 2 elements/cycle with a free per-partition running-min accumulator, so
both engines' PSUM ports stay saturated. Since q is the full d^2, the min is
global: the epilogue is one column-min + relu(TH - x) + cross-partition sum.
The pos-loss prep runs on GpSimd so the V/S queues hold only granule work.
Output per core: [pos_sum, flag]; flag must be exactly 0.
"""

import numpy as np

import concourse.bacc as bacc
import concourse.mybir as mybir
import concourse.tile as tile
from concourse.bass_utils import run_bass_kernel_spmd

N_CORES = 8
N_PTS = 100000
D = 32
P = 16384
M = 8192
P_LOC = P // N_CORES            # 2048 pos-pair rows per core (pos path)
PD = 15360                      # deduped+padded row count (both matrices)
PD_LOC = PD // N_CORES          # 1920 matrix rows per core
RT = PD_LOC // 128              # 15 row tiles per matrix
CHUNK = 1024                    # q columns per PSUM granule (2 banks)
NCH = M // CHUNK                # 8 chunks
KP = D + 1                      # pos-path contraction dim (features + ones)
KM = D + 2                      # matrix contraction dim (+ ones + |p|^2)
POS_THRESH = 0.1
NEG_THRESH = 1.4
TH = 3.0                        # certificate threshold on d^2 (vs
                                # NEG_THRESH^2=1.96; margin covers bf16 error)

F32 = mybir.dt.float32
BF16 = mybir.dt.bfloat16
AX = mybir.AxisListType
ALU = mybir.AluOpType
ACT = mybir.ActivationFunctionType

_CACHED_NC = None
LAST_RESULTS = None            # test.py reads .exec_time_ns after a traced run


def _register_const(nc, value):
    t = nc.alloc_sbuf_tensor(f"const-float32-{value}", [128, 1], F32)
    nc.gpsimd.memset(t.ap(), value)
    nc.const_aps.aps[(F32, value)] = t.ap()


def _register_min2():
    """Custom DVE op: out = min(in0, in1) elementwise, accum_out[p] =
    min(s0, min_k out[p, k]). Consumes TWO streams per cycle (rd0 + rd1),
    doubling reduction throughput vs stock tensor_reduce (which is capped at
    one element/lane/cycle)."""
    import concourse.dve_ops as dops
    from concourse.dve_spec import C0, Spec, Src0, Src1, _has_src1, lower, minn
    from concourse.dve_uop import DveOpSpec

    name = "MIN2_STREAMS_ANT"
    for op in dops.OPS:
        if op.name == name:
            return op

    def ref(in0, in1, s0, s1, imm2):
        b = np.minimum(in0, in1).astype(np.float32)
        acc = np.minimum(b.reshape(b.shape[0], -1).min(-1, keepdims=True),
                         np.asarray(s0, np.float32).reshape(-1, 1))
        return b, acc

    spec = Spec(body=minn(Src0, Src1), accum=minn, accum_init=C0, reference=ref)
    row = dops._CUSTOM_DVE_ROW_BASE + len(dops.OPS)
    shas = {}
    for ver in ("v3", "v4"):
        uops = lower(spec, ver=ver)
        shas[ver] = DveOpSpec(name=name, opcode=row, uops=uops,
                              rd1_en=_has_src1(spec)).sha(ver)
    op = dops.DveOp(name, spec, subdim=False, uops_sha=shas)
    dops.OPS.append(op)
    dops.CUSTOM_DVE_SPECS[name] = spec
    dops._SUB_OPCODE_FOR_NAME[name] = row
    return op


def _build_nc():
    min2 = _register_min2()
    nc = bacc.Bacc("TRN2", debug=False, target_bir_lowering=False,
                   num_devices=N_CORES)
    for v in (-POS_THRESH, TH):
        _register_const(nc, v)
    # fp32 pos-pair operands (positive loss needs full precision);
    # bf16 deduped operands feed the distance-matrix matmuls (fp32 PE matmul
    # streams at 1/4 rate; bf16 error on d^2 is ~0.3 vs a 5+ margin).
    lhsA = nc.dram_tensor("lhsA", [KP, P_LOC], F32, kind="ExternalInput").ap()
    lhsB = nc.dram_tensor("lhsB", [KP, P_LOC], F32, kind="ExternalInput").ap()
    lhsAh = nc.dram_tensor("lhsAh", [KM, PD_LOC], BF16, kind="ExternalInput").ap()
    lhsBh = nc.dram_tensor("lhsBh", [KM, PD_LOC], BF16, kind="ExternalInput").ap()
    rhsAh = nc.dram_tensor("rhsAh", [KM, M], BF16, kind="ExternalInput").ap()
    rhsBh = nc.dram_tensor("rhsBh", [KM, M], BF16, kind="ExternalInput").ap()
    ones = nc.dram_tensor("ones", [128, 1], F32, kind="ExternalInput").ap()
    outd = nc.dram_tensor("out", [1, 2], F32, kind="ExternalOutput").ap()

    with tile.TileContext(nc) as tc:
        with (
            tc.tile_pool(name="ops", bufs=1) as ops,
            tc.tile_pool(name="wk", bufs=2) as wk,
            tc.tile_pool(name="ps", bufs=4, space="PSUM") as ps,
        ):
            t_lhsA = ops.tile([KP, P_LOC], F32, tag="lhsA")
            t_lhsB = ops.tile([KP, P_LOC], F32, tag="lhsB")
            # bf16 operands are loaded TWICE: rows 0..33 and rows 64..97,
            # so the two halves' matmuls run on the two 64-row groups of
            # the PE array with overlapped LDWEIGHTS and back-to-back MM
            # streaming (one shared copy serializes LDW+MM drain: ~520ns
            # per MM, PE-bound at 230us - measured).
            t_lhsAh = ops.tile([128, PD_LOC], BF16, tag="lhsAh")
            t_lhsBh = ops.tile([128, PD_LOC], BF16, tag="lhsBh")
            t_rhsAh = ops.tile([128, M], BF16, tag="rhsAh")
            t_rhsBh = ops.tile([128, M], BF16, tag="rhsBh")
            t_ones = ops.tile([128, 1], F32, tag="ones")
            # per-pair running-min columns, one per global pair index
            t_cmin = ops.tile([128, 128], F32, tag="cmin")

            # memset first on the in-order GpSimd queue (the min2 accum
            # target must be initialized before the first pair completes).
            nc.gpsimd.memset(t_cmin[:], 3.0e38)

            # operand loads strictly in consumption order. Measured ring
            # behavior: the sync ring is a HW DGE with fast completion
            # semaphores but a narrow ~42 GB/s queue set; the gpsimd ring
            # fans wide across queues but its completion sems can lag ~6us
            # while the GpSimd engine is busy generating descriptors; the
            # scalar ring is a 12 GB/s trap. So: everything the first block
            # touches goes FIRST on the sync ring in small pieces (the first
            # matmul only needs lhs cols 0:256 and rhs cols 0:512); the
            # gpsimd ring carries the late-consumed bulk.
            def chunk(t_dst, src, k, base, c0=0, c1=CHUNK):
                sl = slice(k * CHUNK + c0, k * CHUNK + c1)
                return (t_dst[base:base + KM, sl], src[:, sl])

            nc.sync.dma_start(*chunk(t_rhsAh, rhsAh, 0, 0, 0, 512))
            nc.sync.dma_start(t_lhsAh[0:KM, 0:512], lhsAh[:, 0:512])
            nc.sync.dma_start(*chunk(t_rhsAh, rhsAh, 0, 64, 0, 512))
            nc.sync.dma_start(t_lhsAh[64:64 + KM, 0:512], lhsAh[:, 0:512])
            for base in (0, 64):
                nc.sync.dma_start(*chunk(t_rhsAh, rhsAh, 0, base, 512, CHUNK))
            for base in (0, 64):
                nc.sync.dma_start(*chunk(t_rhsAh, rhsAh, 1, base))
            for base in (0, 64):
                nc.sync.dma_start(*chunk(t_rhsAh, rhsAh, 2, base))
            nc.sync.dma_start(t_lhsBh[0:KM, 0:512], lhsBh[:, 0:512])
            nc.sync.dma_start(t_lhsBh[64:64 + KM, 0:512], lhsBh[:, 0:512])
            # gpsimd ring carries the rest (each arrival runs ~7us behind
            # its descriptor issue; issues are ~0.7us apart), in
            # first-consumption order.
            for base in (0, 64):
                nc.gpsimd.dma_start(*chunk(t_rhsAh, rhsAh, 3, base))
            for k in (0, 1, 2, 3):
                for base in (0, 64):
                    nc.gpsimd.dma_start(*chunk(t_rhsBh, rhsBh, k, base))
            for k in (4, 5, 6, 7):
                for base in (0, 64):
                    nc.gpsimd.dma_start(*chunk(t_rhsAh, rhsAh, k, base))
            for k in (4, 5, 6, 7):
                for base in (0, 64):
                    nc.gpsimd.dma_start(*chunk(t_rhsBh, rhsBh, k, base))
            nc.gpsimd.dma_start(t_lhsAh[0:KM, 512:PD_LOC],
                                lhsAh[:, 512:PD_LOC])
            nc.gpsimd.dma_start(t_lhsAh[64:64 + KM, 512:PD_LOC],
                                lhsAh[:, 512:PD_LOC])
            nc.gpsimd.dma_start(t_lhsBh[0:KM, 512:PD_LOC],
                                lhsBh[:, 512:PD_LOC])
            nc.gpsimd.dma_start(t_lhsBh[64:64 + KM, 512:PD_LOC],
                                lhsBh[:, 512:PD_LOC])
            # fp32 pos operands + ones trail on the sync ring; they are only
            # needed by the GpSimd pos-path prep and the tail.
            nc.sync.dma_start(t_lhsA[:], lhsA[:])
            nc.sync.dma_start(t_lhsB[:], lhsB[:])
            nc.sync.dma_start(t_ones[:], ones[:])

            # The ENTIRE positive-pair loss runs on GpSimd (idle otherwise,
            # and it is the one engine that can reduce across partitions),
            # so the V/S queues and the PE hold nothing but granule work:
            # pos_sum = sum_pairs relu(sum_d (p0-p1)^2 - 0.1), written
            # straight into the output tile's first element mid-span.
            from concourse import bass_isa

            t_dif = ops.tile([KP, P_LOC], F32, tag="dif")
            nc.gpsimd.tensor_tensor(t_dif[:], t_lhsA[:], t_lhsB[:],
                                    ALU.subtract)
            t_difsq = ops.tile([KP, P_LOC], F32, tag="difsq")
            nc.gpsimd.tensor_tensor(t_difsq[:], t_dif[:], t_dif[:],
                                    ALU.mult)
            t_psum1 = ops.tile([KP, P_LOC], F32, tag="psum1")
            nc.gpsimd.partition_all_reduce(t_psum1[:], t_difsq[:],
                                           channels=KP,
                                           reduce_op=bass_isa.ReduceOp.add)
            t_posr = ops.tile([1, P_LOC], F32, tag="posr")
            nc.gpsimd.tensor_scalar(t_posr[:], t_psum1[0:1, :],
                                    -POS_THRESH, 0.0, ALU.add, ALU.max)

            t_outsb = wk.tile([1, 2], F32, tag="outsb")
            nc.gpsimd.tensor_reduce(out=t_outsb[0:1, 0:1], in_=t_posr[:],
                                    axis=AX.XYZWC, op=ALU.add)

            # warm the ScalarE activation table during the DMA wait (the
            # first Relu otherwise stalls the copy stream ~1.3us mid-span);
            # reads a const AP that is memset at build start (no DMA dep).
            warm = wk.tile([128, 1], BF16, tag="warm")
            nc.scalar.activation(warm[:], nc.const_aps.aps[(F32, TH)],
                                 ACT.Relu)

            # ---- the two distance matrices ----
            # Row-tiles processed in pairs (PE row-groups 0 and 64); the odd
            # 15th tile runs alone on group 0. ScalarE copies the first
            # granule of each (k, k+1) pair to SBUF (overlapping the second
            # granule's matmuls), then the DVE min2 consumes (odd PSUM, even
            # copy) and SUM^W min-accumulates into the pair's cmin column.
            # q is the FULL d^2 (|p|^2 folded into the matmul), so the min
            # is global and any two granules can form a min2 pair - pairing
            # runs straight through block boundaries with one held copy.
            state = {"held": None, "col": 0}

            def consume(q):
                if state["held"] is None:
                    # first granule of a pair: ScalarE copy to SBUF now;
                    # the copy overlaps the next granule's matmuls.
                    qc = wk.tile([128, CHUNK], F32, tag="qc", bufs=8)
                    nc.scalar.copy(qc[:], q[:])
                    state["held"] = qc
                else:
                    junk = wk.tile([128, CHUNK], F32, tag="junk",
                                   bufs=3)
                    col = state["col"]
                    nc.vector._custom_dve(
                        min2, out=junk[:], in0=q[:],
                        in1=state["held"][:], s0=3.0e38,
                        accum_out=t_cmin[:, col:col + 1])
                    state["held"] = None
                    state["col"] = col + 1

            def granule_pair(sel, k):
                qs = {}
                for half in (0, 1):
                    qs[half] = ps.tile([128, CHUNK], F32, tag="q",
                                       name=f"q{half}")
                # emit the halves' matmuls interleaved by 512-col piece so
                # consecutive PE ops alternate row-groups (lets each
                # LDWEIGHTS pull ahead during the other group's matmul and
                # keeps MMs back-to-back).
                for j in range(CHUNK // 512):
                    for half, (t_lhs, t_rhs, r) in enumerate(sel):
                        base = 64 * half
                        w = t_lhs[base:base + KM, r * 128:(r + 1) * 128]
                        c0 = k * CHUNK + j * 512
                        nc.tensor.matmul(
                            qs[half][:, j * 512:(j + 1) * 512], w,
                            t_rhs[base:base + KM, c0:c0 + 512])
                for half in (0, 1):
                    consume(qs[half])

            # Block-major sweep (weights for a row-tile pair stay loaded
            # across all its chunks - chunk-major reloads weights per pair
            # and measures 25us slower). The FIRST tile-pair is split into
            # 4-chunk half-blocks with A/B interleaved, halving the early
            # DMA demand spike (block 0 otherwise needs the entire 1.1MB
            # A-side rhs within ~9us of span start).
            sched = [(0, 0, 0, 4), (0, 1, 0, 4), (1, 0, 0, 4),
                     (1, 1, 0, 4), (0, 0, 4, 8), (0, 1, 4, 8),
                     (1, 0, 4, 8), (1, 1, 4, 8)]
            sched += [(pr, mi, 0, 8) for pr in range(2, RT // 2)
                      for mi in (0, 1)]
            sched += [(RT // 2, 0, 0, 8)]
            for pr, mi, k0, k1 in sched:
                if pr == RT // 2:
                    sel = ((t_lhsAh, t_rhsAh, RT - 1),
                           (t_lhsBh, t_rhsBh, RT - 1))
                elif mi == 0:
                    sel = ((t_lhsAh, t_rhsAh, 2 * pr),
                           (t_lhsAh, t_rhsAh, 2 * pr + 1))
                else:
                    sel = ((t_lhsBh, t_rhsBh, 2 * pr),
                           (t_lhsBh, t_rhsBh, 2 * pr + 1))
                for k in range(k0, k1):
                    granule_pair(sel, k)

            # ---- epilogue: global min -> relu(TH - min) must be 0, summed
            # across partitions with a ones-matmul.
            m1 = wk.tile([128, 1], F32, tag="m1")
            nc.vector.tensor_reduce(out=m1[:], in_=t_cmin[:], axis=AX.X,
                                    op=ALU.min)
            junkq = wk.tile([128, 1], BF16, tag="junkq")
            t_flag = wk.tile([128, 1], F32, tag="flag")
            nc.scalar.activation(junkq[:], m1[:], ACT.Relu, bias=TH,
                                 scale=-1.0, accum_out=t_flag[:])
            fp = ps.tile([1, 1], F32, tag="q")
            nc.tensor.matmul(fp[:], t_flag[:], t_ones[:])
            nc.scalar.copy(t_outsb[0:1, 1:2], fp[0:1, 0:1])

            nc.sync.dma_start(outd[:], t_outsb[:])

    nc.compile()
    return nc


def _dedup_pad(F, idx):
    """F[unique(idx)] padded with zero rows to PD; returns None if the
    unique count exceeds the static padding (host fallback then)."""
    u = np.unique(np.asarray(idx))
    if u.size > PD:
        return None
    out = np.zeros((PD, D), np.float32)
    out[:u.size] = F[u]
    return out


def _prep_inputs(F0, F1, matches, sel0, sel1):
    posF0 = F0[matches[:, 0]]
    posF1 = F1[matches[:, 1]]
    subF0 = F0[sel0]
    subF1 = F1[sel1]
    posF0u = _dedup_pad(F0, matches[:, 0])
    posF1u = _dedup_pad(F1, matches[:, 1])
    if posF0u is None or posF1u is None:
        return None
    import ml_dtypes

    bf16 = ml_dtypes.bfloat16

    def rhs_aug(sub):
        return np.ascontiguousarray(np.concatenate(
            [-2.0 * sub.T, (sub * sub).sum(1)[None, :],
             np.ones((1, M), np.float32)], 0), dtype=bf16)

    def lhs_aug(p):
        return np.ascontiguousarray(np.concatenate(
            [p.T, np.ones((1, p.shape[0]), np.float32),
             (p * p).sum(1)[None, :]], 0), dtype=bf16)

    rhsAh = rhs_aug(subF1)
    rhsBh = rhs_aug(subF0)
    ones_col = np.ones((1, P_LOC), np.float32)
    ones_in = np.ones((128, 1), np.float32)
    in_maps = []
    for c in range(N_CORES):
        sl = slice(c * P_LOC, (c + 1) * P_LOC)
        sld = slice(c * PD_LOC, (c + 1) * PD_LOC)
        in_maps.append({
            "lhsA": np.ascontiguousarray(
                np.concatenate([posF0[sl].T, ones_col], 0), dtype=np.float32),
            "lhsB": np.ascontiguousarray(
                np.concatenate([posF1[sl].T, ones_col], 0), dtype=np.float32),
            "lhsAh": lhs_aug(posF0u[sld]),
            "lhsBh": lhs_aug(posF1u[sld]),
            "rhsAh": rhsAh,
            "rhsBh": rhsBh,
            "ones": ones_in,
        })
    return in_maps


def _exact_host_reference(F0, F1, matches, sel0, sel1):
    """Bit-faithful numpy port of the oracle, used only as a fallback when a
    nonzero hardest-negative sum is observed (mask handling then matters)."""
    hash_seed = max(F0.shape[0], F1.shape[0])
    pos_ind0 = matches[:, 0].astype(np.int64)
    pos_ind1 = matches[:, 1].astype(np.int64)
    posF0, posF1 = F0[pos_ind0], F1[pos_ind1]
    subF0, subF1 = F0[sel0], F1[sel1]

    def pd(A, B):
        d2 = ((A * A).sum(1)[:, None] + (B * B).sum(1)[None, :]
              - 2.0 * (A @ B.T))
        return np.sqrt(np.maximum(d2, 0.0) + 1e-7)

    D01 = pd(posF0, subF1)
    D10 = pd(posF1, subF0)
    D01min, D10min = D01.min(1), D10.min(1)
    D01ind = np.asarray(sel1)[np.argmin(D01, 1)].astype(np.int64)
    D10ind = np.asarray(sel0)[np.argmin(D10, 1)].astype(np.int64)
    pos_keys = pos_ind0 + pos_ind1 * hash_seed
    mask0 = ~np.isin(pos_ind0 + D01ind * hash_seed, pos_keys)
    mask1 = ~np.isin(D10ind + pos_ind1 * hash_seed, pos_keys)
    pos_loss = np.mean(np.maximum(((posF0 - posF1) ** 2).sum(1) - POS_THRESH, 0))
    n0 = np.maximum(NEG_THRESH - D01min, 0) ** 2
    n1 = np.maximum(NEG_THRESH - D10min, 0) ** 2
    neg0 = (n0 * mask0).sum() / max(mask0.sum(), 1)
    neg1 = (n1 * mask1).sum() / max(mask1.sum(), 1)
    return np.float32(pos_loss + (neg0 + neg1) / 2.0)


def kernel(F0, F1, matches, sel0, sel1):
    global _CACHED_NC, LAST_RESULTS
    F0 = np.ascontiguousarray(np.asarray(F0), dtype=np.float32)
    F1 = np.ascontiguousarray(np.asarray(F1), dtype=np.float32)
    matches = np.asarray(matches)
    sel0 = np.asarray(sel0)
    sel1 = np.asarray(sel1)
    assert F0.shape == (N_PTS, D) and matches.shape == (P, 2)
    assert sel0.shape == (M,) and sel1.shape == (M,)

    in_maps = _prep_inputs(F0, F1, matches, sel0, sel1)
    if in_maps is None:
        # more unique rows than the static padding allows
        return _exact_host_reference(F0, F1, matches, sel0, sel1)
    if _CACHED_NC is None:
        _CACHED_NC = _build_nc()
    try:
        res = run_bass_kernel_spmd(_CACHED_NC, in_maps, list(range(N_CORES)))
    except Exception:
        # a wedged NeuronCore (e.g. NRT_EXEC_UNIT_UNRECOVERABLE from an
        # earlier crashed session) is recoverable via the axon reset call
        try:
            import ctypes

            lib = ctypes.CDLL("/opt/axon/libaxon_pjrt.so")
            lib.axon_reset.restype = ctypes.c_int64
            lib.axon_reset()
        except Exception:
            pass
        res = run_bass_kernel_spmd(_CACHED_NC, in_maps, list(range(N_CORES)))
    LAST_RESULTS = res
    outs = np.stack([r["out"] for r in res.results])   # (8, 1, 2)
    pos_sum = float(outs[:, 0, 0].sum())
    flag = float(outs[:, 0, 1].sum())
    if flag != 0.0:
        # some distance crossed the certificate threshold: the hardest-
        # negative terms / pair-mask now matter; recompute exactly on host.
        return _exact_host_reference(F0, F1, matches, sel0, sel1)
    return np.float32(pos_sum / P)
